# revision 1
# baseline (speedup 1.0000x reference)
"""Trainium2 Bass kernel for the style-modulated encoder layer.

Per batch sample b (data-parallel over B=8 across 8 cores):
  styles = w @ (affine_weight/sqrt(512)).T + affine_bias        [1024]
  s1, s2 = styles[:512], styles[512:]
  xm = x * s1;  xn = instance_norm(xm) over hidden dim (eps=1e-5)
  qd/kd/vd = rsqrt(sum_h (W*s1)^2 + 1e-8); wd likewise with s2
  q = (xn @ qW.T)*qd; k = (xn @ kW.T)*kd; v = (xn @ vW.T)*vd*s2
  o = softmax(q k^T / sqrt(32)) v   (16 heads, depth 32)
  o = (o @ wW.T)*wd + noise_const*noise_strength + bias
  o = leaky_relu(o, 0.2); clip(o, +-256)

Layout strategy (per core):
  x loaded natural [s, h]; instance-norm via bn_stats over the free dim;
  xn transposed on PE to [h, s] for the projections.  q, k computed
  transposed [o, s]; v computed natural [s, o] (operand swap).  Attention
  uses transposed scores [k_s, q_s] so exp'd probs feed the attn@v matmul
  directly; softmax row-sums come from ones-matmuls using a [128,32] ones
  lhsT (result pre-broadcast over each head's 32 partitions).  Division by
  the row-sum is applied to the small attn output via exp(-ln(rowsum)).
  Output projection contracts over h giving natural [s, o] tiles that DMA
  straight out.
"""

import numpy as np

S = 1024
H = 512
P = 128
HT = H // P          # 4 h-tiles
ST = S // P          # 8 s-tiles
NHEADS = 16
DEPTH = 32
NG = 4               # head groups of 4 heads (= o-tiles)
QB = 512             # q-block (free dim of transposed scores)
NQB = S // QB        # attention q-blocks
PB = 512             # projection free-dim block
NPB = S // PB
SCALE = DEPTH ** -0.5
CLAMP = 256.0
N_CORES = 8

_F32R = True         # matmul operands viewed as float32r (full-rate fp32)


def _build(nc, mybir, bass, tile, stage=99, nreps=1):
    f32 = mybir.dt.float32
    f32r = mybir.dt.float32r
    bf16 = mybir.dt.bfloat16
    Alu = mybir.AluOpType
    Act = mybir.ActivationFunctionType
    from concourse.masks import make_identity

    def r(ap):
        return ap

    # ---- DRAM I/O ----
    x_d = nc.dram_tensor("x", [S, H], f32, kind="ExternalInput")
    w_d = nc.dram_tensor("w", [1, H], f32, kind="ExternalInput")
    aw_d = nc.dram_tensor("affine_weight", [2 * H, H], f32, kind="ExternalInput")
    ab_d = nc.dram_tensor("affine_bias", [2 * H], f32, kind="ExternalInput")
    qw_d = nc.dram_tensor("q_weight", [H, H], f32, kind="ExternalInput")
    kw_d = nc.dram_tensor("k_weight", [H, H], f32, kind="ExternalInput")
    vw_d = nc.dram_tensor("v_weight", [H, H], f32, kind="ExternalInput")
    ww_d = nc.dram_tensor("w_weight", [H, H], f32, kind="ExternalInput")
    ncst_d = nc.dram_tensor("noise_const", [S, 1], f32, kind="ExternalInput")
    ns_d = nc.dram_tensor("noise_strength", [1, 1], f32, kind="ExternalInput")
    bias_d = nc.dram_tensor("bias", [1, H], f32, kind="ExternalInput")
    out_d = nc.dram_tensor("out", [S, H], f32, kind="ExternalOutput")

    def bcast_row(dram_ap, n, offset_elems=0):
        # [n] contiguous DRAM -> [128, n] partition-broadcast read AP
        return bass.AP(
            tensor=dram_ap.tensor,
            offset=dram_ap.offset + offset_elems,
            ap=[[0, P], [1, n]],
        )

    def col_ap(dram_ap, ncols, offset_elems=0):
        # flat DRAM -> [128, ncols]; (p, c) = v[c*128 + p]
        return bass.AP(
            tensor=dram_ap.tensor,
            offset=dram_ap.offset + offset_elems,
            ap=[[1, P], [P, ncols]],
        )

    with tile.TileContext(nc) as tc:
        with (
            tc.tile_pool(name="persist", bufs=1) as pp,
            tc.tile_pool(name="wtp", bufs=2) as wtp,
            tc.tile_pool(name="work", bufs=3) as wp,
            tc.tile_pool(name="expp", bufs=3) as ep,
            tc.tile_pool(name="psA", bufs=2, space="PSUM") as psA,
            tc.tile_pool(name="psB", bufs=1, space="PSUM") as psB,
            tc.tile_pool(name="dram", bufs=1, space="DRAM") as dp,
        ):
          for _rep in range(nreps):
            # ---------------- constants / small loads ----------------
            ident = pp.tile([P, P], f32, tag="ident")
            make_identity(nc, ident)

            ones32 = pp.tile([P, DEPTH], bf16, tag="ones32")
            nc.vector.memset(ones32, 1.0)

            eps_n = pp.tile([P, 1], f32, tag="eps_n")
            nc.vector.memset(eps_n, 1e-5)
            eps_d = pp.tile([P, 1], f32, tag="eps_d")
            nc.vector.memset(eps_d, 1e-8)

            w_bc = pp.tile([P, H], f32, tag="w_bc")
            nc.gpsimd.dma_start(out=w_bc, in_=bcast_row(w_d[:], H))

            ab_col = pp.tile([P, 8], f32, tag="ab_col")
            nc.gpsimd.dma_start(out=ab_col, in_=col_ap(ab_d[:], 8))

            noise_col = pp.tile([P, ST], f32, tag="noise_col")
            nc.gpsimd.dma_start(out=noise_col, in_=col_ap(ncst_d[:], ST))
            ns_col = pp.tile([P, 1], f32, tag="ns_col")
            nc.gpsimd.dma_start(out=ns_col, in_=bcast_row(ns_d[:], 1))
            nc.vector.tensor_scalar(noise_col, noise_col, ns_col, None, Alu.mult)

            bias_bc = pp.tile([P, H], f32, tag="bias_bc")
            nc.gpsimd.dma_start(out=bias_bc, in_=bcast_row(bias_d[:], H))

            # ---------------- styles ----------------
            styles_col = pp.tile([P, 8], f32, tag="styles_col")
            for t in range(8):
                aw_t = wp.tile([P, H], f32, tag="aw_t")
                nc.sync.dma_start(out=aw_t, in_=aw_d[t * P:(t + 1) * P, :])
                scr = wp.tile([P, H], f32, tag="scr")
                nc.vector.tensor_tensor(scr, aw_t, w_bc, Alu.mult)
                red = wp.tile([P, 1], f32, tag="red")
                nc.vector.tensor_reduce(
                    out=red, in_=scr, axis=mybir.AxisListType.X, op=Alu.add
                )
                # styles = sum/sqrt(H) + affine_bias
                nc.vector.tensor_scalar(
                    styles_col[:, t:t + 1], red,
                    1.0 / float(np.sqrt(H)), ab_col[:, t:t + 1],
                    Alu.mult, Alu.add,
                )
            s2_col = styles_col[:, 4:8]

            # roundtrip scratch: styles col form -> row form -> bcast tiles
            scratch = dp.tile([4 * H], f32, tag="scratch")
            nc.gpsimd.dma_start(out=col_ap(scratch[:], 8), in_=styles_col)
            s1_bc = pp.tile([P, H], f32, tag="s1_bc")
            nc.gpsimd.dma_start(out=s1_bc, in_=bcast_row(scratch[:], H, 0))
            s2_bc = pp.tile([P, H], f32, tag="s2_bc")
            nc.gpsimd.dma_start(out=s2_bc, in_=bcast_row(scratch[:], H, H))

            if stage <= 1:
                nc.sync.dma_start(out=out_d[0:P, :], in_=s1_bc)
                return nc

            # ---------------- x: modulate + instance norm + transpose ------
            xnT = pp.tile([P, HT, S], f32r, tag="xnT")
            for st in range(ST):
                x_t = wp.tile([P, H], f32, tag="x_t")
                nc.sync.dma_start(out=x_t, in_=x_d[st * P:(st + 1) * P, :])
                nc.vector.tensor_tensor(x_t, x_t, s1_bc, Alu.mult)
                stats = wp.tile([P, 6], f32, tag="bn_stats")
                nc.vector.bn_stats(out=stats, in_=x_t)
                mv = wp.tile([P, 2], f32, tag="bn_mv")
                nc.vector.bn_aggr(out=mv, in_=stats)
                # rstd = 1/sqrt(var+eps) = exp(-0.5*ln(var+eps))
                nc.scalar.activation(
                    out=mv[:, 1:2], in_=mv[:, 1:2], func=Act.Ln, bias=eps_n
                )
                nc.scalar.activation(
                    out=mv[:, 1:2], in_=mv[:, 1:2], func=Act.Exp, scale=-0.5
                )
                nc.vector.tensor_scalar(
                    x_t, x_t, mv[:, 0:1], mv[:, 1:2], Alu.subtract, Alu.mult
                )
                for hc in range(HT):
                    tp = psA.tile([P, P], f32, tag="ps_s")
                    nc.tensor.transpose(tp, x_t[:, hc * P:(hc + 1) * P], ident)
                    nc.vector.tensor_copy(
                        out=xnT[:, hc, st * P:(st + 1) * P], in_=tp
                    )

            if stage <= 2:
                xv = wp.tile([P, H], f32, tag="xv")
                nc.vector.tensor_copy(out=xv, in_=xnT[:, 0, 0:H])
                nc.sync.dma_start(out=out_d[0:P, :], in_=xv)
                return nc

            # ------------- weights: load + demod + transpose + project -----
            dall = pp.tile([P, 16], f32, tag="dall")  # raw demod sums
            q_sb = pp.tile([P, NG, S], f32r, tag="q_sb")
            k_sb = pp.tile([P, NG, S], f32r, tag="k_sb")
            v_sb = pp.tile([P, ST, H], bf16, tag="v_sb")
            wT_w = None  # output-projection weight, kept until the end

            for wi, (name, wsrc) in enumerate(
                [("q", qw_d), ("k", kw_d), ("v", vw_d), ("w", ww_d)]
            ):
                s_bc = s2_bc if name == "w" else s1_bc
                wT_sb = wtp.tile([P, HT, H], f32r, tag="wT")
                for ot in range(HT):
                    w_n = wp.tile([P, H], f32, tag="w_nat")
                    nc.sync.dma_start(out=w_n, in_=wsrc[ot * P:(ot + 1) * P, :])
                    ws = wp.tile([P, H], f32, tag="scr")
                    nc.vector.tensor_tensor(ws, w_n, s_bc, Alu.mult)
                    sq = wp.tile([P, H], f32, tag="sq_scr")
                    nc.scalar.activation(
                        out=sq, in_=ws, func=Act.Square,
                        accum_out=dall[:, wi * 4 + ot: wi * 4 + ot + 1],
                    )
                    for hc in range(HT):
                        tp = psA.tile([P, P], f32, tag="ps_s")
                        nc.tensor.transpose(tp, w_n[:, hc * P:(hc + 1) * P], ident)
                        nc.vector.tensor_copy(
                            out=wT_sb[:, hc, ot * P:(ot + 1) * P], in_=tp
                        )

                # rsqrt of this weight's demod sums: exp(-0.5*ln(x+1e-8))
                dcol = pp.tile([P, 4], f32, tag=f"dcol_{name}")
                nc.scalar.activation(
                    out=dcol, in_=dall[:, wi * 4:wi * 4 + 4], func=Act.Ln, bias=eps_d
                )
                nc.scalar.activation(out=dcol, in_=dcol, func=Act.Exp, scale=-0.5)

                if name in ("q", "k"):
                    dst = q_sb if name == "q" else k_sb
                    for ot in range(NG):
                        for sb in range(NPB):
                            ps = psA.tile([P, PB], f32, tag="ps_s")
                            for ht in range(HT):
                                nc.tensor.matmul(
                                    ps,
                                    r(wT_sb[:, ht, ot * P:(ot + 1) * P]),
                                    r(xnT[:, ht, sb * PB:(sb + 1) * PB]),
                                    start=(ht == 0), stop=(ht == HT - 1),
                                )
                            nc.vector.tensor_scalar(
                                dst[:, ot, sb * PB:(sb + 1) * PB], ps,
                                dcol[:, ot:ot + 1], None, Alu.mult,
                            )
                elif name == "v":
                    # vds2 row-broadcast: vd (col) * s2 (col) -> scratch -> row
                    vds2_col = pp.tile([P, 4], f32, tag="vds2_col")
                    nc.vector.tensor_tensor(vds2_col, dcol, s2_col, Alu.mult)
                    nc.gpsimd.dma_start(
                        out=col_ap(scratch[:], 4, 2 * H), in_=vds2_col
                    )
                    vds2_bc = pp.tile([P, H], f32, tag="vds2_bc")
                    nc.gpsimd.dma_start(
                        out=vds2_bc, in_=bcast_row(scratch[:], H, 2 * H)
                    )
                    for st in range(ST):
                        ps = psA.tile([P, PB], f32, tag="ps_s")
                        for ht in range(HT):
                            nc.tensor.matmul(
                                ps[:, :H],
                                r(xnT[:, ht, st * P:(st + 1) * P]),
                                r(wT_sb[:, ht, :]),
                                start=(ht == 0), stop=(ht == HT - 1),
                            )
                        nc.vector.tensor_tensor(
                            v_sb[:, st, :], ps[:, :H], vds2_bc, Alu.mult
                        )
                else:  # "w"
                    wT_w = wT_sb
                    nc.gpsimd.dma_start(out=col_ap(scratch[:], 4, 3 * H), in_=dcol)
                    wdr_bc = pp.tile([P, H], f32, tag="wdr_bc")
                    nc.gpsimd.dma_start(
                        out=wdr_bc, in_=bcast_row(scratch[:], H, 3 * H)
                    )

            if stage <= 3:
                qv = wp.tile([P, H], f32, tag="xv")
                nc.vector.tensor_copy(out=qv, in_=q_sb[:, 0, 0:H])
                nc.sync.dma_start(out=out_d[0:P, :], in_=qv)
                return nc

            # ---------------- attention ----------------
            # o_ps / rs_ps accumulate 4 col-packed heads x 8 k-tiles in one
            # PSUM group per bank.  The group is opened by a full-width K=1
            # zero-matmul (start=True over all 128 partitions) and closed by a
            # zero-accumulate (stop=True), with an explicit dep chain pinning
            # the order (PSUM group tracking is partition-blind per bank).
            from concourse.bass import _add_dep_helper

            zrow = pp.tile([1, P], bf16, tag="zrow")
            nc.vector.memset(zrow, 0.0)
            zrhs = pp.tile([1, QB], bf16, tag="zrhs")
            nc.vector.memset(zrhs, 0.0)

            oT = pp.tile([P, NG, S], f32r, tag="oT")
            for g in range(NG):
                for qb in range(NQB):
                    sc_ps = psB.tile([P, 4 * QB], f32, tag="sc_ps")
                    o_ps = psB.tile([P, QB], f32, tag="o_ps")
                    rs_ps = psB.tile([P, QB], f32, tag="rs_ps")
                    chains = {"o": [], "rs": []}

                    def mm(which, *args, **kwargs):
                        inst = nc.tensor.matmul(*args, **kwargs)
                        ch = chains[which]
                        if ch:
                            _add_dep_helper(
                                inst.ins, ch[-1].ins, sync=False,
                                reason="psum bank group order",
                            )
                        ch.append(inst)

                    mm("o", o_ps, r(zrow), r(zrhs), start=True, stop=False)
                    mm("rs", rs_ps, r(zrow), r(zrhs), start=True, stop=False)
                    for kt in range(ST):
                        expt = ep.tile([P, 4 * QB], bf16, tag="expt")
                        # half-exps (2 heads each) so PE work overlaps ACT
                        for half in range(2):
                            for j in (2 * half, 2 * half + 1):
                                nc.tensor.matmul(
                                    sc_ps[:, j * QB:(j + 1) * QB],
                                    r(k_sb[32 * j:32 * (j + 1), g, kt * P:(kt + 1) * P]),
                                    r(q_sb[32 * j:32 * (j + 1), g, qb * QB:(qb + 1) * QB]),
                                    start=True, stop=True,
                                    tile_position=(32 * j, 0),
                                )
                            nc.scalar.activation(
                                out=expt[:, 2 * half * QB:(2 * half + 2) * QB],
                                in_=sc_ps[:, 2 * half * QB:(2 * half + 2) * QB],
                                func=Act.Exp, scale=SCALE,
                            )
                        for j in range(4):
                            mm(
                                "o",
                                o_ps[32 * j:32 * (j + 1), :],
                                r(v_sb[:, kt, g * P + 32 * j: g * P + 32 * (j + 1)]),
                                r(expt[:, j * QB:(j + 1) * QB]),
                                start=False, stop=False,
                                tile_position=(0, 32 * j),
                            )
                            mm(
                                "rs",
                                rs_ps[32 * j:32 * (j + 1), :],
                                r(ones32),
                                r(expt[:, j * QB:(j + 1) * QB]),
                                start=False, stop=False,
                                tile_position=(0, 32 * j),
                            )
                    mm("o", o_ps, r(zrow), r(zrhs), start=False, stop=True)
                    mm("rs", rs_ps, r(zrow), r(zrhs), start=False, stop=True)

                    # o / rowsum  via exp(-ln(rowsum))
                    rs_rec = wp.tile([P, QB], f32, tag="rs_rec")
                    nc.scalar.activation(out=rs_rec, in_=rs_ps, func=Act.Ln)
                    nc.scalar.activation(
                        out=rs_rec, in_=rs_rec, func=Act.Exp, scale=-1.0
                    )
                    nc.vector.tensor_tensor(
                        oT[:, g, qb * QB:(qb + 1) * QB], o_ps, rs_rec, Alu.mult
                    )

            if stage <= 4:
                ov = wp.tile([P, H], f32, tag="xv")
                nc.vector.tensor_copy(out=ov, in_=oT[:, 0, 0:H])
                nc.sync.dma_start(out=out_d[0:P, :], in_=ov)
                return nc

            # ---------------- output projection + epilogue ----------------
            for st in range(ST):
                ps = psA.tile([P, PB], f32, tag="ps_s")
                for g in range(NG):
                    nc.tensor.matmul(
                        ps[:, :H],
                        r(oT[:, g, st * P:(st + 1) * P]),
                        r(wT_w[:, g, :]),
                        start=(g == 0), stop=(g == NG - 1),
                    )
                t1 = wp.tile([P, H], f32, tag="ep_t1")
                nc.vector.tensor_tensor(t1, ps[:, :H], wdr_bc, Alu.mult)
                nc.vector.tensor_scalar(
                    t1, t1, noise_col[:, st:st + 1], None, Alu.add
                )
                nc.vector.tensor_tensor(t1, t1, bias_bc, Alu.add)
                t2 = wp.tile([P, H], f32, tag="ep_t2")
                # leaky_relu(0.2) = max(x, 0.2x)
                nc.vector.tensor_scalar(t2, t1, 0.2, None, Alu.mult)
                nc.vector.tensor_tensor(t2, t1, t2, Alu.max)
                nc.vector.tensor_scalar(t2, t2, CLAMP, -CLAMP, Alu.min, Alu.max)
                nc.sync.dma_start(out=out_d[st * P:(st + 1) * P, :], in_=t2)

    return nc


def build_bass(stage=99, nreps=1):
    import concourse.bass as bass
    import concourse.bacc as bacc
    import concourse.mybir as mybir
    import concourse.tile as tile

    nc = bacc.Bacc()
    _build(nc, mybir, bass, tile, stage, nreps)
    nc.compile()
    return nc


def make_in_map(inputs, b):
    return {
        "x": np.ascontiguousarray(inputs["x"][b], np.float32),
        "w": np.ascontiguousarray(inputs["w"][b:b + 1], np.float32),
        "affine_weight": np.ascontiguousarray(inputs["affine_weight"], np.float32),
        "affine_bias": np.ascontiguousarray(inputs["affine_bias"], np.float32),
        "q_weight": np.ascontiguousarray(inputs["q_weight"], np.float32),
        "k_weight": np.ascontiguousarray(inputs["k_weight"], np.float32),
        "v_weight": np.ascontiguousarray(inputs["v_weight"], np.float32),
        "w_weight": np.ascontiguousarray(inputs["w_weight"], np.float32),
        "noise_const": np.ascontiguousarray(inputs["noise_const"], np.float32),
        "noise_strength": np.asarray(inputs["noise_strength"], np.float32).reshape(1, 1),
        "bias": np.asarray(inputs["bias"], np.float32).reshape(1, H),
    }


def kernel(**inputs):
    from concourse.bass_utils import run_bass_kernel_spmd

    nc = build_bass()
    in_maps = [make_in_map(inputs, b) for b in range(N_CORES)]
    res = run_bass_kernel_spmd(nc, in_maps, core_ids=list(range(N_CORES)))
    out = np.stack([res.results[b]["out"] for b in range(N_CORES)], axis=0)
    return out.astype(np.float32)

